# revision 22
# baseline (speedup 1.0000x reference)
"""DeepseekV3 MLA attention (B=1, S=2048, D=2048, H=16) on 8 trn2 NeuronCores.

Strategy (tensor-parallel over heads, replicated low-rank projections):
  - every core computes the full q_a / kv_a low-rank projections (+rmsnorm)
    from a host-transposed hidden state, entirely in a "transposed" layout
    (feature dim on partitions, sequence on the free dim) so attention
    operands come out pre-transposed for the PE;
  - each core owns 2 heads: it computes q_b / kv_b for them, causal
    flash-style attention (no max subtraction -- logits are O(1) here), and
    its slice of o_proj, producing a partial [S, D] output;
  - host sums the 8 partials.

All matmuls run in bf16 (fp32 PSUM accumulation); rmsnorm stats, rope and
softmax run in fp32.  RoPE deinterleave + rotate-half are folded into the
weight layout on the host (extra "pre-swapped, sign-folded" weight columns)
so the device only does aligned elementwise mul/adds.
"""

import numpy as np
import ml_dtypes

import concourse.bass as bass
import concourse.mybir as mybir
import concourse.tile as tile
from concourse.bass_utils import run_bass_kernel_spmd

BF16 = ml_dtypes.bfloat16
F32 = mybir.dt.float32
BF = mybir.dt.bfloat16

B, S, D = 1, 2048, 2048
H = 16
N_CORES = 8
HPC = H // N_CORES  # heads per core = 2
Q_LORA = 1536
KV_LORA = 512
NOPE = 128
ROPE = 64
VD = 128
QHD = NOPE + ROPE  # 192
THETA = 50000.0
EPS = 1e-6
SCALE = QHD ** (-0.5)

NQ = 512            # q-chunk (matmul free dim)
NCHUNK = S // NQ    # 4
KT = S // 128       # 16 k-tiles
AF = mybir.ActivationFunctionType

LAST_RESULTS = None
_CACHE = {}


# ----------------------------------------------------------------------------
# host-side weight preparation
# ----------------------------------------------------------------------------

def _deint_perm():
    # deinterleave: out[j] = in[2j] (j<32), in[2(j-32)+1] (j>=32)
    p = np.empty(ROPE, dtype=np.int64)
    p[:32] = 2 * np.arange(32)
    p[32:] = 2 * np.arange(32) + 1
    return p


def _rope_tables(position_ids):
    pos = np.asarray(position_ids).reshape(-1).astype(np.float32)  # [S]
    inv_freq = (1.0 / (THETA ** (np.arange(0, ROPE, 2, dtype=np.float32) / ROPE)))
    freqs = np.outer(pos, inv_freq)  # [S, 32]
    cos32 = np.cos(freqs).T.astype(np.float32)  # [32, S]
    sin32 = np.sin(freqs).T.astype(np.float32)
    cos128 = np.tile(cos32, (4, 1))  # [128, S]
    sin128 = np.tile(sin32, (4, 1))
    return cos128, sin128


def _causal_mask_big():
    # M[dk, u] = 1 if u >= dk + 384 ; slice [:, 384-128*i : 896-128*i]
    # gives the diagonal-block mask indicator(dq >= dk + 128*i)
    dk = np.arange(128)[:, None]
    u = np.arange(1024)[None, :]
    return (u >= dk + 384).astype(BF16)


def _prep_inputs(inputs):
    hidden = np.asarray(inputs["hidden_states"], dtype=np.float32)[0]  # [S, D]
    position_ids = np.asarray(inputs["position_ids"])
    q_a_w = np.asarray(inputs["q_a_w"], dtype=np.float32)        # [1536, D]
    q_a_ln_w = np.asarray(inputs["q_a_ln_w"], dtype=np.float32)  # [1536]
    q_b_w = np.asarray(inputs["q_b_w"], dtype=np.float32)        # [H*192, 1536]
    kv_a_w = np.asarray(inputs["kv_a_w"], dtype=np.float32)      # [576, D]
    kv_a_ln_w = np.asarray(inputs["kv_a_ln_w"], dtype=np.float32)  # [512]
    kv_b_w = np.asarray(inputs["kv_b_w"], dtype=np.float32)      # [H*256, 512]
    o_w = np.asarray(inputs["o_w"], dtype=np.float32)            # [D, H*128]

    dp = _deint_perm()
    dps = dp[(np.arange(ROPE) ^ 32)]          # source index for the swapped term
    sgn = np.where(np.arange(ROPE) < 32, -1.0, 1.0).astype(np.float32)[:, None]

    shared = {}
    shared["hT"] = np.ascontiguousarray(hidden.T).astype(BF16)          # [D, S]
    shared["qaT"] = np.ascontiguousarray(q_a_w.T).astype(BF16)          # [D, 1536]

    # kv_a columns: [ckv 512 | kpe 64 (deint) | kpe2 64 (swap+sign)]
    kva_cols = np.concatenate(
        [kv_a_w[:KV_LORA], kv_a_w[KV_LORA + dp], sgn * kv_a_w[KV_LORA + dps]], axis=0
    )  # [640, D]
    shared["kvaT"] = np.ascontiguousarray(kva_cols.T).astype(BF16)      # [D, 640]

    cos128, sin128 = _rope_tables(position_ids)
    shared["cosb"] = cos128
    shared["sinb"] = sin128
    shared["maskb"] = _causal_mask_big()

    # q_b with ln + scale folded
    qb = q_b_w * q_a_ln_w[None, :] * SCALE  # [H*192, 1536]
    qb = qb.reshape(H, QHD, Q_LORA)
    kvb = (kv_b_w * kv_a_ln_w[None, :]).reshape(H, NOPE + VD, KV_LORA)

    per_core = []
    for c in range(N_CORES):
        h0, h1 = HPC * c, HPC * c + 1
        nope0 = qb[h0, :NOPE]            # [128, 1536]
        nope1 = qb[h1, :NOPE]
        peP = np.concatenate([qb[h0, NOPE + dp], qb[h1, NOPE + dp]], axis=0)  # [128,...]
        pe2P = np.concatenate(
            [sgn * qb[h0, NOPE + dps], sgn * qb[h1, NOPE + dps]], axis=0
        )
        qb_cols = np.concatenate([nope0, nope1, peP, pe2P], axis=0)  # [512, 1536]
        kb_cols = np.concatenate([kvb[h0, :NOPE], kvb[h1, :NOPE]], axis=0)  # [256, 512]
        vb_cols = np.concatenate([kvb[h0, NOPE:], kvb[h1, NOPE:]], axis=0)  # [256, 512]
        o_slice = o_w[:, VD * h0 : VD * (h1 + 1)]  # [D, 256]
        per_core.append(
            {
                "qbT": np.ascontiguousarray(qb_cols.T).astype(BF16),   # [1536, 512]
                "kbT": np.ascontiguousarray(kb_cols.T).astype(BF16),   # [512, 256]
                "vbT": np.ascontiguousarray(vb_cols.T).astype(BF16),   # [512, 256]
                "owT": np.ascontiguousarray(o_slice.T).astype(BF16),   # [256, S... D]
            }
        )
    return shared, per_core


# ----------------------------------------------------------------------------
# numpy simulation of the device program (for host-side validation)
# ----------------------------------------------------------------------------

def _sim_core(shared, pc):
    bf = lambda x: x.astype(BF16).astype(np.float32)
    hT = shared["hT"].astype(np.float32)          # [D, S]
    qaT = shared["qaT"].astype(np.float32)        # [D, 1536]
    kvaT = shared["kvaT"].astype(np.float32)      # [D, 640]
    cos = shared["cosb"]                          # [128, S]
    sin = shared["sinb"]
    qbT = pc["qbT"].astype(np.float32)            # [1536, 512]
    kbT = pc["kbT"].astype(np.float32)            # [512, 256]
    vbT = pc["vbT"].astype(np.float32)            # [512, 256]
    owT = pc["owT"].astype(np.float32)            # [256, D]

    qaTx = qaT.T @ hT                             # [1536, S]
    qaTb = bf(qaTx)                               # bf16 copy used downstream
    ssq = (bf(qaTb * qaTb)).sum(axis=0)           # square in bf16, fp32 sum
    inv = 1.0 / np.sqrt(ssq / Q_LORA + EPS)       # [S]
    qT = qbT.T @ qaTb                             # [512, S]
    qn0 = bf(qT[0:128] * inv)
    qn1 = bf(qT[128:256] * inv)
    pe, pe2 = qT[256:384], qT[384:512]
    qpe = bf((pe * cos + pe2 * sin) * inv)        # [128, S] packed (h0;h1)

    ckvT = kvaT.T @ hT                            # [640, S]
    ckv = ckvT[:KV_LORA]
    ckvb = bf(ckv)
    ssc = (bf(ckvb * ckvb)).sum(axis=0)
    invc = 1.0 / np.sqrt(ssc / KV_LORA + EPS)
    ckvn = bf(ckvb * invc)                        # [512, S]
    kpe, kpe2 = ckvT[512:576], ckvT[576:640]
    kper = bf(kpe * cos[0:64] + kpe2 * sin[0:64])  # [64, S]

    out = np.zeros((S, D), dtype=np.float32)
    for j in range(HPC):
        knT = bf(kbT[:, 128 * j : 128 * (j + 1)].T @ ckvn)   # [128, S]
        v = bf(ckvn.T @ vbT[:, 128 * j : 128 * (j + 1)])     # [S, 128]
        qn = qn0 if j == 0 else qn1
        qp = qpe[64 * j : 64 * (j + 1)]
        scores = knT.T @ qn + kper.T @ qp         # [S(k), S(q)] -> st[k, q]
        st = scores
        kidx = np.arange(S)[:, None]
        qidx = np.arange(S)[None, :]
        p = np.exp(st) * (kidx <= qidx)
        p = bf(p)
        rs = p.sum(axis=0)                        # [q]
        oT = (v.T @ p)                            # [128, q]
        oT = bf(oT * (1.0 / rs))
        out += oT.T @ owT[128 * j : 128 * (j + 1)]
    return out


def sim(inputs):
    shared, per_core = _prep_inputs(inputs)
    out = np.zeros((S, D), dtype=np.float32)
    for c in range(N_CORES):
        out += _sim_core(shared, per_core[c])
    return out.reshape(B, S, D)


# ----------------------------------------------------------------------------
# bass program
# ----------------------------------------------------------------------------

def _split_waits(nc, max_waits=1):
    """This walrus build accepts at most one sem wait per instruction; hoist
    excess waits onto pure-wait EventSemaphore carriers just before it."""
    n_new = 0
    for f in nc.m.functions:
        for blk in f.blocks:
            new_insts = []
            for inst in blk.instructions:
                si = getattr(inst, "sync_info", None)
                waits = list(si.on_wait) if (si is not None and si.on_wait) else []
                if len(waits) > max_waits:
                    extra, keep = waits[:-max_waits], waits[-max_waits:]
                    for w in extra:
                        n_new += 1
                        carrier = mybir.InstEventSemaphore(
                            name=f"ws-{n_new}-{inst.name}",
                            engine=inst.engine,
                            ins=[],
                            outs=[],
                            sync_info=mybir.SyncInfo(on_wait=[w], on_update=[]),
                        )
                        nc.register_instruction(carrier, overwrite=True)
                        new_insts.append(carrier)
                    si.on_wait = keep
                new_insts.append(inst)
            blk.instructions = new_insts
    return n_new


def _build_nc():
    nc = bass.Bass()
    hT = nc.dram_tensor("hT", [D, S], BF, kind="ExternalInput")
    qaT = nc.dram_tensor("qaT", [D, Q_LORA], BF, kind="ExternalInput")
    kvaT = nc.dram_tensor("kvaT", [D, 640], BF, kind="ExternalInput")
    qbT = nc.dram_tensor("qbT", [Q_LORA, 512], BF, kind="ExternalInput")
    kbT = nc.dram_tensor("kbT", [KV_LORA, 256], BF, kind="ExternalInput")
    vbT = nc.dram_tensor("vbT", [KV_LORA, 256], BF, kind="ExternalInput")
    owT = nc.dram_tensor("owT", [2 * VD, D], BF, kind="ExternalInput")
    cosb = nc.dram_tensor("cosb", [128, S], F32, kind="ExternalInput")
    sinb = nc.dram_tensor("sinb", [128, S], F32, kind="ExternalInput")
    maskb = nc.dram_tensor("maskb", [128, 1024], BF, kind="ExternalInput")
    out = nc.dram_tensor("out", [S, D], F32, kind="ExternalOutput")

    QL_T = Q_LORA // 128  # 12
    D_T = D // 128        # 16
    CV_T = KV_LORA // 128  # 4

    with tile.TileContext(nc) as tc:
        with tc.tile_pool(name="persist1", bufs=1) as persist1:
            ones_t = persist1.tile([128, 128], BF, tag="ones")
            eps_t = persist1.tile([128, 1], F32, tag="eps")
            nc.vector.memset(eps_t, EPS)
            nc.vector.memset(ones_t, 1.0)
            qn_T = [persist1.tile([128, S], BF, tag=f"qnT{h}", name=f"qnT{h}") for h in range(HPC)]
            qpeP = persist1.tile([128, S], BF, tag="qpeP")
            qpe1 = persist1.tile([64, S], BF, tag="qpe1")
            ckvn = [persist1.tile([128, S], BF, tag=f"ckvn{i}", name=f"ckvn{i}") for i in range(CV_T)]
            kper = persist1.tile([64, S], BF, tag="kper")

            # ------------- merged stage 1: q & kv paths, one hidden pass -------------
            with tc.tile_pool(name="qaw", bufs=1) as qaw, \
                 tc.tile_pool(name="kvw", bufs=1) as kvw, \
                 tc.tile_pool(name="qbw", bufs=1) as qbw, \
                 tc.tile_pool(name="hx", bufs=2) as hx, \
                 tc.tile_pool(name="qasb", bufs=1) as qasb, \
                 tc.tile_pool(name="cvsb", bufs=1) as cvsb, \
                 tc.tile_pool(name="csp", bufs=2) as csp, \
                 tc.tile_pool(name="sq", bufs=2) as sqp, \
                 tc.tile_pool(name="nrm", bufs=2) as nrm, \
                 tc.tile_pool(name="nrm2", bufs=2) as nrm2, \
                 tc.tile_pool(name="pet", bufs=1) as pet, \
                 tc.tile_pool(name="st_ps", bufs=3, space="PSUM") as st_ps, \
                 tc.tile_pool(name="ssq_ps", bufs=1, space="PSUM") as ssq_ps, \
                 tc.tile_pool(name="ssq2_ps", bufs=1, space="PSUM") as ssq2_ps, \
                 tc.tile_pool(name="qt_ps", bufs=3, space="PSUM") as qt_ps:

                qa_w = qaw.tile([128, D_T, Q_LORA], BF, tag="qaw")
                kva_w = kvw.tile([128, D_T, 640], BF, tag="kvw")
                qb_w = qbw.tile([128, QL_T, 512], BF, tag="qbw")
                for k in range(D_T):
                    nc.sync.dma_start(out=kva_w[:, k, 0:128], in_=kvaT[128 * k : 128 * (k + 1), 0:128])

                for c in range(NCHUNK):
                    cs = slice(NQ * c, NQ * (c + 1))
                    h_t = hx.tile([128, D_T, NQ], BF, tag="h")
                    for k in range(D_T):
                        nc.sync.dma_start(out=h_t[:, k, :], in_=hT[128 * k : 128 * (k + 1), cs])
                    cos_c = csp.tile([128, NQ], F32, tag="cosc")
                    sin_c = csp.tile([128, NQ], F32, tag="sinc")
                    nc.sync.dma_start(out=cos_c, in_=cosb[:, cs])
                    nc.sync.dma_start(out=sin_c, in_=sinb[:, cs])
                    if c == 0:
                        for k in range(D_T):
                            nc.sync.dma_start(out=kva_w[:, k, 128:640], in_=kvaT[128 * k : 128 * (k + 1), 128:640])
                        for k in range(D_T):
                            nc.sync.dma_start(out=qa_w[:, k, :], in_=qaT[128 * k : 128 * (k + 1), :])
                        for m in range(QL_T):
                            nc.sync.dma_start(out=qb_w[:, m, :], in_=qbT[128 * m : 128 * (m + 1), :])

                    # ---- kv_a: 4 ckv m-tiles + kpe + kpe2 ----
                    cv_t = cvsb.tile([128, CV_T, NQ], BF, tag="cv")
                    ssc = ssq2_ps.tile([128, NQ], F32, tag="ssc")
                    pe_ps = []
                    for m in range(6):
                        mp = 128 if m < 4 else 64
                        col = slice(128 * m, 128 * m + 128) if m < 4 else \
                            slice(512 + 64 * (m - 4), 512 + 64 * (m - 3))
                        ps = st_ps.tile([mp, NQ], F32, tag="stps")
                        for k in range(D_T):
                            nc.tensor.matmul(
                                ps,
                                kva_w[:, k, col],
                                h_t[:, k, :],
                                start=(k == 0),
                                stop=(k == D_T - 1),
                            )
                        if m < 4:
                            nc.vector.tensor_copy(cv_t[:, m, :], ps)
                            sq = sqp.tile([128, NQ], BF, tag="sq")
                            nc.scalar.activation(out=sq, in_=ps, func=AF.Square)
                            nc.tensor.matmul(
                                ssc, ones_t, sq, start=(m == 0), stop=(m == CV_T - 1)
                            )
                        else:
                            pe_ps.append(ps)

                    # ---- q_a: 12 m-tiles ----
                    qa_t = qasb.tile([128, QL_T, NQ], BF, tag="qa")
                    ssq = ssq_ps.tile([128, NQ], F32, tag="ssq")
                    for m in range(QL_T):
                        ps = st_ps.tile([128, NQ], F32, tag="stps")
                        for k in range(D_T):
                            nc.tensor.matmul(
                                ps,
                                qa_w[:, k, 128 * m : 128 * (m + 1)],
                                h_t[:, k, :],
                                start=(k == 0),
                                stop=(k == D_T - 1),
                            )
                        nc.vector.tensor_copy(qa_t[:, m, :], ps)
                        sq = sqp.tile([128, NQ], BF, tag="sq")
                        nc.scalar.activation(out=sq, in_=ps, func=AF.Square)
                        nc.tensor.matmul(
                            ssq, ones_t, sq, start=(m == 0), stop=(m == QL_T - 1)
                        )

                    # ---- kv norm + kpe rope ----
                    bc2 = nrm2.tile([128, NQ], F32, tag="bc2")
                    nc.scalar.activation(
                        out=bc2, in_=ssc, func=AF.Sqrt, scale=1.0 / KV_LORA, bias=eps_t
                    )
                    nc.vector.reciprocal(bc2, bc2)
                    for i in range(CV_T):
                        nc.vector.tensor_mul(ckvn[i][:, cs], cv_t[:, i, :], bc2)
                    t1 = pet.tile([128, NQ], F32, tag="t1")
                    t2 = pet.tile([128, NQ], F32, tag="t2")
                    nc.vector.tensor_mul(t1[0:64, :], pe_ps[0], cos_c[0:64, :])
                    nc.vector.tensor_mul(t2[0:64, :], pe_ps[1], sin_c[0:64, :])
                    nc.vector.tensor_add(kper[:, cs], t1[0:64, :], t2[0:64, :])

                    # ---- q_b: 4 col-blocks accumulated over 12 m ----
                    bc = nrm.tile([128, NQ], F32, tag="bc")
                    nc.scalar.activation(
                        out=bc, in_=ssq, func=AF.Sqrt, scale=1.0 / Q_LORA, bias=eps_t
                    )
                    nc.vector.reciprocal(bc, bc)
                    qt_tiles = []
                    for b in range(4):
                        ps = qt_ps.tile([128, NQ], F32, tag="qtps")
                        for m in range(QL_T):
                            nc.tensor.matmul(
                                ps,
                                qb_w[:, m, 128 * b : 128 * (b + 1)],
                                qa_t[:, m, :],
                                start=(m == 0),
                                stop=(m == QL_T - 1),
                            )
                        if b == 0:
                            nc.vector.tensor_mul(qn_T[0][:, cs], ps, bc)
                        elif b == 1:
                            nc.vector.tensor_mul(qn_T[1][:, cs], ps, bc)
                        else:
                            qt_tiles.append(ps)
                    nc.vector.tensor_mul(t1, qt_tiles[0], cos_c)
                    nc.vector.tensor_mul(t2, qt_tiles[1], sin_c)
                    nc.vector.tensor_add(t1, t1, t2)
                    nc.vector.tensor_mul(qpeP[:, cs], t1, bc)
                nc.sync.dma_start(out=qpe1[:, :], in_=qpeP[64:128, :])

            # ---------------- phase B2: kv_b projections ----------------
            with tc.tile_pool(name="persist2", bufs=1) as persist2:
                kn_T = [persist2.tile([128, S], BF, tag=f"knT{h}", name=f"knT{h}") for h in range(HPC)]
                v_sb = [persist2.tile([128, S], BF, tag=f"v{h}", name=f"v{h}") for h in range(HPC)]
                o_T = [persist2.tile([128, S], BF, tag=f"oT{h}", name=f"oT{h}") for h in range(HPC)]
                with tc.tile_pool(name="kbw", bufs=1) as kbw, \
                     tc.tile_pool(name="kn_ps", bufs=2, space="PSUM") as kn_ps, \
                     tc.tile_pool(name="v_ps", bufs=3, space="PSUM") as v_ps:
                    kb_w = kbw.tile([128, CV_T, 256], BF, tag="kbw")
                    vb_w = kbw.tile([128, CV_T, 256], BF, tag="vbw")
                    for ct in range(CV_T):
                        nc.sync.dma_start(out=kb_w[:, ct, :], in_=kbT[128 * ct : 128 * (ct + 1), :])
                        nc.sync.dma_start(out=vb_w[:, ct, :], in_=vbT[128 * ct : 128 * (ct + 1), :])
                    for h in range(HPC):
                        hs = slice(128 * h, 128 * (h + 1))
                        for c in range(NCHUNK):
                            cs = slice(NQ * c, NQ * (c + 1))
                            ps = kn_ps.tile([128, NQ], F32, tag="knps")
                            for ct in range(CV_T):
                                nc.tensor.matmul(
                                    ps,
                                    kb_w[:, ct, hs],
                                    ckvn[ct][:, cs],
                                    start=(ct == 0),
                                    stop=(ct == CV_T - 1),
                                )
                            nc.vector.tensor_copy(kn_T[h][:, cs], ps)
                        for kt in range(KT):
                            ks = slice(128 * kt, 128 * (kt + 1))
                            ps = v_ps.tile([128, VD], F32, tag="vps")
                            for ct in range(CV_T):
                                nc.tensor.matmul(
                                    ps,
                                    ckvn[ct][:, ks],
                                    vb_w[:, ct, hs],
                                    start=(ct == 0),
                                    stop=(ct == CV_T - 1),
                                )
                            nc.vector.tensor_copy(v_sb[h][:, ks], ps)

                # ---------------- phase C: attention ----------------
                mskp_cm = tc.tile_pool(name="mskp", bufs=1)
                oww_cm = tc.tile_pool(name="oww", bufs=1)
                mskp = mskp_cm.__enter__()
                oww = oww_cm.__enter__()
                with tc.tile_pool(name="pp", bufs=4) as pp, \
                     tc.tile_pool(name="ep", bufs=2) as ep, \
                     tc.tile_pool(name="rvp", bufs=2) as rvp, \
                     tc.tile_pool(name="ostg", bufs=4) as ostg, \
                     tc.tile_pool(name="s_ps", bufs=3, space="PSUM") as s_ps, \
                     tc.tile_pool(name="rs_ps", bufs=2, space="PSUM") as rs_ps, \
                     tc.tile_pool(name="o_ps", bufs=2, space="PSUM") as o_ps, \
                     tc.tile_pool(name="out_ps", bufs=1, space="PSUM") as out_ps:
                    mask_s = mskp.tile([128, 1024], BF, tag="mask")
                    nc.sync.dma_start(out=mask_s, in_=maskb[:, :])
                    ow_t = oww.tile([128, HPC, D], BF, tag="oww")
                    for j in range(HPC):
                        nc.sync.dma_start(out=ow_t[:, j, :], in_=owT[128 * j : 128 * (j + 1), :])
                    for c in range(NCHUNK):
                        cs = slice(NQ * c, NQ * (c + 1))
                        nkt = 4 * (c + 1)
                        for h in range(HPC):
                            qpe_h = qpeP[0:64, :] if h == 0 else qpe1
                            rs = rs_ps.tile([128, NQ], F32, tag="rs")
                            op = o_ps.tile([128, NQ], F32, tag="op")
                            for kt in range(nkt):
                                ks = slice(128 * kt, 128 * (kt + 1))
                                i = kt - 4 * c
                                lo = 128 * i if i > 0 else 0  # valid q-subrange start
                                qs = slice(NQ * c + lo, NQ * (c + 1))
                                vs = slice(lo, NQ)
                                sp = s_ps.tile([128, NQ], F32, tag="sp")
                                nc.tensor.matmul(
                                    sp[:, vs], kn_T[h][:, ks], qn_T[h][:, qs],
                                    start=True, stop=False,
                                )
                                nc.tensor.matmul(
                                    sp[:, vs], kper[:, ks], qpe_h[:, qs],
                                    start=False, stop=True,
                                )
                                p_t = pp.tile([128, NQ], BF, tag="p")
                                if kt >= 4 * c:
                                    e_t = ep.tile([128, NQ], BF, tag="e")
                                    nc.scalar.activation(out=e_t[:, vs], in_=sp[:, vs], func=AF.Exp)
                                    nc.vector.tensor_mul(
                                        p_t[:, vs], e_t[:, vs],
                                        mask_s[:, 384 : 896 - lo],
                                    )
                                else:
                                    nc.scalar.activation(out=p_t[:, vs], in_=sp[:, vs], func=AF.Exp)
                                nc.tensor.matmul(
                                    rs[:, vs], ones_t, p_t[:, vs],
                                    start=(kt == 0), stop=(kt == nkt - 1),
                                )
                                nc.tensor.matmul(
                                    op[:, vs],
                                    v_sb[h][:, ks],
                                    p_t[:, vs],
                                    start=(kt == 0), stop=(kt == nkt - 1),
                                )
                            rv = rvp.tile([128, NQ], F32, tag="rv")
                            nc.vector.reciprocal(rv, rs)
                            nc.vector.tensor_mul(o_T[h][:, cs], op, rv)
                        # o_proj for this chunk's 4 s-tiles (both heads now done;
                        # last chunk handled in a post-phase with deeper PSUM)
                        for si in range(4 * c, 4 * (c + 1) if c < NCHUNK - 1 else 4 * c):
                            ss = slice(128 * si, 128 * (si + 1))
                            for nch in range(NCHUNK):
                                ns = slice(NQ * nch, NQ * (nch + 1))
                                ps = out_ps.tile([128, NQ], F32, tag="outps")
                                for j in range(HPC):
                                    nc.tensor.matmul(
                                        ps,
                                        o_T[j][:, ss],
                                        ow_t[:, j, ns],
                                        start=(j == 0),
                                        stop=(j == HPC - 1),
                                    )
                                stg = ostg.tile([128, NQ], F32, tag="ostg")
                                nc.scalar.activation(out=stg, in_=ps, func=AF.Copy)
                                nc.sync.dma_start(out=out[ss, ns], in_=stg)
                # ---------------- final chunk o_proj ----------------
                with tc.tile_pool(name="ostg2", bufs=4) as ostg2, \
                     tc.tile_pool(name="out2_ps", bufs=4, space="PSUM") as out2_ps:
                    for si in range(4 * (NCHUNK - 1), 4 * NCHUNK):
                        ss = slice(128 * si, 128 * (si + 1))
                        for nch in range(NCHUNK):
                            ns = slice(NQ * nch, NQ * (nch + 1))
                            ps = out2_ps.tile([128, NQ], F32, tag="out2ps")
                            for j in range(HPC):
                                nc.tensor.matmul(
                                    ps,
                                    o_T[j][:, ss],
                                    ow_t[:, j, ns],
                                    start=(j == 0),
                                    stop=(j == HPC - 1),
                                )
                            stg = ostg2.tile([128, NQ], F32, tag="ostg2")
                            nc.scalar.activation(out=stg, in_=ps, func=AF.Copy)
                            nc.sync.dma_start(out=out[ss, ns], in_=stg)
                oww_cm.__exit__(None, None, None)
                mskp_cm.__exit__(None, None, None)

    _split_waits(nc)
    return nc


# ----------------------------------------------------------------------------
# entry point
# ----------------------------------------------------------------------------

def kernel(**inputs):
    global LAST_RESULTS
    shared, per_core = _prep_inputs(inputs)
    if "nc" not in _CACHE:
        _CACHE["nc"] = _build_nc()
    nc = _CACHE["nc"]
    in_maps = []
    for c in range(N_CORES):
        m = {
            "hT": shared["hT"],
            "qaT": shared["qaT"],
            "kvaT": shared["kvaT"],
            "cosb": shared["cosb"],
            "sinb": shared["sinb"],
            "maskb": shared["maskb"],
            "qbT": per_core[c]["qbT"],
            "kbT": per_core[c]["kbT"],
            "vbT": per_core[c]["vbT"],
            "owT": per_core[c]["owT"],
        }
        in_maps.append(m)
    res = run_bass_kernel_spmd(nc, in_maps, core_ids=list(range(N_CORES)))
    LAST_RESULTS = res
    out = np.zeros((S, D), dtype=np.float32)
    for r in res.results:
        out += r["out"]
    return out.reshape(B, S, D)


# revision 23
# speedup vs baseline: 1.0167x; 1.0167x over previous
"""DeepseekV3 MLA attention (B=1, S=2048, D=2048, H=16) on 8 trn2 NeuronCores.

Strategy (tensor-parallel over heads, replicated low-rank projections):
  - every core computes the full q_a / kv_a low-rank projections (+rmsnorm)
    from a host-transposed hidden state, entirely in a "transposed" layout
    (feature dim on partitions, sequence on the free dim) so attention
    operands come out pre-transposed for the PE;
  - each core owns 2 heads: it computes q_b / kv_b for them, causal
    flash-style attention (no max subtraction -- logits are O(1) here), and
    its slice of o_proj, producing a partial [S, D] output;
  - host sums the 8 partials.

All matmuls run in bf16 (fp32 PSUM accumulation); rmsnorm stats, rope and
softmax run in fp32.  RoPE deinterleave + rotate-half are folded into the
weight layout on the host (extra "pre-swapped, sign-folded" weight columns)
so the device only does aligned elementwise mul/adds.
"""

import numpy as np
import ml_dtypes

import concourse.bass as bass
import concourse.mybir as mybir
import concourse.tile as tile
from concourse.bass_utils import run_bass_kernel_spmd

BF16 = ml_dtypes.bfloat16
F32 = mybir.dt.float32
BF = mybir.dt.bfloat16

B, S, D = 1, 2048, 2048
H = 16
N_CORES = 8
HPC = H // N_CORES  # heads per core = 2
Q_LORA = 1536
KV_LORA = 512
NOPE = 128
ROPE = 64
VD = 128
QHD = NOPE + ROPE  # 192
THETA = 50000.0
EPS = 1e-6
SCALE = QHD ** (-0.5)

NQ = 512            # q-chunk (matmul free dim)
NCHUNK = S // NQ    # 4
KT = S // 128       # 16 k-tiles
AF = mybir.ActivationFunctionType

LAST_RESULTS = None
_CACHE = {}


# ----------------------------------------------------------------------------
# host-side weight preparation
# ----------------------------------------------------------------------------

def _deint_perm():
    # deinterleave: out[j] = in[2j] (j<32), in[2(j-32)+1] (j>=32)
    p = np.empty(ROPE, dtype=np.int64)
    p[:32] = 2 * np.arange(32)
    p[32:] = 2 * np.arange(32) + 1
    return p


def _rope_tables(position_ids):
    pos = np.asarray(position_ids).reshape(-1).astype(np.float32)  # [S]
    inv_freq = (1.0 / (THETA ** (np.arange(0, ROPE, 2, dtype=np.float32) / ROPE)))
    freqs = np.outer(pos, inv_freq)  # [S, 32]
    cos32 = np.cos(freqs).T.astype(np.float32)  # [32, S]
    sin32 = np.sin(freqs).T.astype(np.float32)
    cos128 = np.tile(cos32, (4, 1))  # [128, S]
    sin128 = np.tile(sin32, (4, 1))
    return cos128, sin128


def _causal_mask_big():
    # M[dk, u] = 1 if u >= dk + 384 ; slice [:, 384-128*i : 896-128*i]
    # gives the diagonal-block mask indicator(dq >= dk + 128*i)
    dk = np.arange(128)[:, None]
    u = np.arange(1024)[None, :]
    return (u >= dk + 384).astype(BF16)


def _prep_inputs(inputs):
    hidden = np.asarray(inputs["hidden_states"], dtype=np.float32)[0]  # [S, D]
    position_ids = np.asarray(inputs["position_ids"])
    q_a_w = np.asarray(inputs["q_a_w"], dtype=np.float32)        # [1536, D]
    q_a_ln_w = np.asarray(inputs["q_a_ln_w"], dtype=np.float32)  # [1536]
    q_b_w = np.asarray(inputs["q_b_w"], dtype=np.float32)        # [H*192, 1536]
    kv_a_w = np.asarray(inputs["kv_a_w"], dtype=np.float32)      # [576, D]
    kv_a_ln_w = np.asarray(inputs["kv_a_ln_w"], dtype=np.float32)  # [512]
    kv_b_w = np.asarray(inputs["kv_b_w"], dtype=np.float32)      # [H*256, 512]
    o_w = np.asarray(inputs["o_w"], dtype=np.float32)            # [D, H*128]

    dp = _deint_perm()
    dps = dp[(np.arange(ROPE) ^ 32)]          # source index for the swapped term
    sgn = np.where(np.arange(ROPE) < 32, -1.0, 1.0).astype(np.float32)[:, None]

    shared = {}
    shared["hT"] = np.ascontiguousarray(hidden.T).astype(BF16)          # [D, S]
    shared["qaT"] = np.ascontiguousarray(q_a_w.T).astype(BF16)          # [D, 1536]

    # kv_a columns: [ckv 512 | kpe 64 (deint) | kpe2 64 (swap+sign)]
    kva_cols = np.concatenate(
        [kv_a_w[:KV_LORA], kv_a_w[KV_LORA + dp], sgn * kv_a_w[KV_LORA + dps]], axis=0
    )  # [640, D]
    shared["kvaT"] = np.ascontiguousarray(kva_cols.T).astype(BF16)      # [D, 640]

    cos128, sin128 = _rope_tables(position_ids)
    shared["cosb"] = cos128
    shared["sinb"] = sin128
    shared["maskb"] = _causal_mask_big()

    # q_b with ln + scale folded
    qb = q_b_w * q_a_ln_w[None, :] * SCALE  # [H*192, 1536]
    qb = qb.reshape(H, QHD, Q_LORA)
    kvb = (kv_b_w * kv_a_ln_w[None, :]).reshape(H, NOPE + VD, KV_LORA)

    per_core = []
    for c in range(N_CORES):
        h0, h1 = HPC * c, HPC * c + 1
        nope0 = qb[h0, :NOPE]            # [128, 1536]
        nope1 = qb[h1, :NOPE]
        peP = np.concatenate([qb[h0, NOPE + dp], qb[h1, NOPE + dp]], axis=0)  # [128,...]
        pe2P = np.concatenate(
            [sgn * qb[h0, NOPE + dps], sgn * qb[h1, NOPE + dps]], axis=0
        )
        qb_cols = np.concatenate([nope0, nope1, peP, pe2P], axis=0)  # [512, 1536]
        kb_cols = np.concatenate([kvb[h0, :NOPE], kvb[h1, :NOPE]], axis=0)  # [256, 512]
        vb_cols = np.concatenate([kvb[h0, NOPE:], kvb[h1, NOPE:]], axis=0)  # [256, 512]
        o_slice = o_w[:, VD * h0 : VD * (h1 + 1)]  # [D, 256]
        per_core.append(
            {
                "qbT": np.ascontiguousarray(qb_cols.T).astype(BF16),   # [1536, 512]
                "kbT": np.ascontiguousarray(kb_cols.T).astype(BF16),   # [512, 256]
                "vbT": np.ascontiguousarray(vb_cols.T).astype(BF16),   # [512, 256]
                "owT": np.ascontiguousarray(o_slice.T).astype(BF16),   # [256, S... D]
            }
        )
    return shared, per_core


# ----------------------------------------------------------------------------
# numpy simulation of the device program (for host-side validation)
# ----------------------------------------------------------------------------

def _sim_core(shared, pc):
    bf = lambda x: x.astype(BF16).astype(np.float32)
    hT = shared["hT"].astype(np.float32)          # [D, S]
    qaT = shared["qaT"].astype(np.float32)        # [D, 1536]
    kvaT = shared["kvaT"].astype(np.float32)      # [D, 640]
    cos = shared["cosb"]                          # [128, S]
    sin = shared["sinb"]
    qbT = pc["qbT"].astype(np.float32)            # [1536, 512]
    kbT = pc["kbT"].astype(np.float32)            # [512, 256]
    vbT = pc["vbT"].astype(np.float32)            # [512, 256]
    owT = pc["owT"].astype(np.float32)            # [256, D]

    qaTx = qaT.T @ hT                             # [1536, S]
    qaTb = bf(qaTx)                               # bf16 copy used downstream
    ssq = (bf(qaTb * qaTb)).sum(axis=0)           # square in bf16, fp32 sum
    inv = 1.0 / np.sqrt(ssq / Q_LORA + EPS)       # [S]
    qT = qbT.T @ qaTb                             # [512, S]
    qn0 = bf(qT[0:128] * inv)
    qn1 = bf(qT[128:256] * inv)
    pe, pe2 = qT[256:384], qT[384:512]
    qpe = bf((pe * cos + pe2 * sin) * inv)        # [128, S] packed (h0;h1)

    ckvT = kvaT.T @ hT                            # [640, S]
    ckv = ckvT[:KV_LORA]
    ckvb = bf(ckv)
    ssc = (bf(ckvb * ckvb)).sum(axis=0)
    invc = 1.0 / np.sqrt(ssc / KV_LORA + EPS)
    ckvn = bf(ckvb * invc)                        # [512, S]
    kpe, kpe2 = ckvT[512:576], ckvT[576:640]
    kper = bf(kpe * cos[0:64] + kpe2 * sin[0:64])  # [64, S]

    out = np.zeros((S, D), dtype=np.float32)
    for j in range(HPC):
        knT = bf(kbT[:, 128 * j : 128 * (j + 1)].T @ ckvn)   # [128, S]
        v = bf(ckvn.T @ vbT[:, 128 * j : 128 * (j + 1)])     # [S, 128]
        qn = qn0 if j == 0 else qn1
        qp = qpe[64 * j : 64 * (j + 1)]
        scores = knT.T @ qn + kper.T @ qp         # [S(k), S(q)] -> st[k, q]
        st = scores
        kidx = np.arange(S)[:, None]
        qidx = np.arange(S)[None, :]
        p = np.exp(st) * (kidx <= qidx)
        p = bf(p)
        rs = p.sum(axis=0)                        # [q]
        oT = (v.T @ p)                            # [128, q]
        oT = bf(oT * (1.0 / rs))
        out += oT.T @ owT[128 * j : 128 * (j + 1)]
    return out


def sim(inputs):
    shared, per_core = _prep_inputs(inputs)
    out = np.zeros((S, D), dtype=np.float32)
    for c in range(N_CORES):
        out += _sim_core(shared, per_core[c])
    return out.reshape(B, S, D)


# ----------------------------------------------------------------------------
# bass program
# ----------------------------------------------------------------------------

def _split_waits(nc, max_waits=1):
    """This walrus build accepts at most one sem wait per instruction; hoist
    excess waits onto pure-wait EventSemaphore carriers just before it."""
    n_new = 0
    for f in nc.m.functions:
        for blk in f.blocks:
            new_insts = []
            for inst in blk.instructions:
                si = getattr(inst, "sync_info", None)
                waits = list(si.on_wait) if (si is not None and si.on_wait) else []
                if len(waits) > max_waits:
                    extra, keep = waits[:-max_waits], waits[-max_waits:]
                    for w in extra:
                        n_new += 1
                        carrier = mybir.InstEventSemaphore(
                            name=f"ws-{n_new}-{inst.name}",
                            engine=inst.engine,
                            ins=[],
                            outs=[],
                            sync_info=mybir.SyncInfo(on_wait=[w], on_update=[]),
                        )
                        nc.register_instruction(carrier, overwrite=True)
                        new_insts.append(carrier)
                    si.on_wait = keep
                new_insts.append(inst)
            blk.instructions = new_insts
    return n_new


def _build_nc():
    nc = bass.Bass()
    hT = nc.dram_tensor("hT", [D, S], BF, kind="ExternalInput")
    qaT = nc.dram_tensor("qaT", [D, Q_LORA], BF, kind="ExternalInput")
    kvaT = nc.dram_tensor("kvaT", [D, 640], BF, kind="ExternalInput")
    qbT = nc.dram_tensor("qbT", [Q_LORA, 512], BF, kind="ExternalInput")
    kbT = nc.dram_tensor("kbT", [KV_LORA, 256], BF, kind="ExternalInput")
    vbT = nc.dram_tensor("vbT", [KV_LORA, 256], BF, kind="ExternalInput")
    owT = nc.dram_tensor("owT", [2 * VD, D], BF, kind="ExternalInput")
    cosb = nc.dram_tensor("cosb", [128, S], F32, kind="ExternalInput")
    sinb = nc.dram_tensor("sinb", [128, S], F32, kind="ExternalInput")
    maskb = nc.dram_tensor("maskb", [128, 1024], BF, kind="ExternalInput")
    out = nc.dram_tensor("out", [S, D], F32, kind="ExternalOutput")

    QL_T = Q_LORA // 128  # 12
    D_T = D // 128        # 16
    CV_T = KV_LORA // 128  # 4

    with tile.TileContext(nc) as tc:
        with tc.tile_pool(name="persist1", bufs=1) as persist1:
            ones_t = persist1.tile([128, 128], BF, tag="ones")
            eps_t = persist1.tile([128, 1], F32, tag="eps")
            nc.vector.memset(eps_t, EPS)
            nc.vector.memset(ones_t, 1.0)
            qn_T = [persist1.tile([128, S], BF, tag=f"qnT{h}", name=f"qnT{h}") for h in range(HPC)]
            qpeP = persist1.tile([128, S], BF, tag="qpeP")
            qpe1 = persist1.tile([64, S], BF, tag="qpe1")
            ckvn = [persist1.tile([128, S], BF, tag=f"ckvn{i}", name=f"ckvn{i}") for i in range(CV_T)]
            kper = persist1.tile([64, S], BF, tag="kper")

            # ------------- merged stage 1: q & kv paths, one hidden pass -------------
            with tc.tile_pool(name="qaw", bufs=1) as qaw, \
                 tc.tile_pool(name="kvw", bufs=1) as kvw, \
                 tc.tile_pool(name="qbw", bufs=1) as qbw, \
                 tc.tile_pool(name="hx", bufs=2) as hx, \
                 tc.tile_pool(name="qasb", bufs=1) as qasb, \
                 tc.tile_pool(name="cvsb", bufs=1) as cvsb, \
                 tc.tile_pool(name="csp", bufs=2) as csp, \
                 tc.tile_pool(name="sq", bufs=2) as sqp, \
                 tc.tile_pool(name="nrm", bufs=2) as nrm, \
                 tc.tile_pool(name="nrm2", bufs=2) as nrm2, \
                 tc.tile_pool(name="pet", bufs=1) as pet, \
                 tc.tile_pool(name="st_ps", bufs=3, space="PSUM") as st_ps, \
                 tc.tile_pool(name="ssq_ps", bufs=1, space="PSUM") as ssq_ps, \
                 tc.tile_pool(name="ssq2_ps", bufs=1, space="PSUM") as ssq2_ps, \
                 tc.tile_pool(name="qt_ps", bufs=3, space="PSUM") as qt_ps:

                qa_w = qaw.tile([128, D_T, Q_LORA], BF, tag="qaw")
                kva_w = kvw.tile([128, D_T, 640], BF, tag="kvw")
                qb_w = qbw.tile([128, QL_T, 512], BF, tag="qbw")
                for k in range(D_T):
                    nc.sync.dma_start(out=kva_w[:, k, :], in_=kvaT[128 * k : 128 * (k + 1), :])

                for c in range(NCHUNK):
                    cs = slice(NQ * c, NQ * (c + 1))
                    h_t = hx.tile([128, D_T, NQ], BF, tag="h")
                    for k in range(D_T):
                        nc.sync.dma_start(out=h_t[:, k, :], in_=hT[128 * k : 128 * (k + 1), cs])
                    cos_c = csp.tile([128, NQ], F32, tag="cosc")
                    sin_c = csp.tile([128, NQ], F32, tag="sinc")
                    nc.sync.dma_start(out=cos_c, in_=cosb[:, cs])
                    nc.sync.dma_start(out=sin_c, in_=sinb[:, cs])
                    if c == 0:
                        for k in range(D_T):
                            nc.sync.dma_start(out=qa_w[:, k, :], in_=qaT[128 * k : 128 * (k + 1), :])
                        for m in range(QL_T):
                            nc.sync.dma_start(out=qb_w[:, m, :], in_=qbT[128 * m : 128 * (m + 1), :])

                    # ---- kv_a: 4 ckv m-tiles + kpe + kpe2 ----
                    cv_t = cvsb.tile([128, CV_T, NQ], BF, tag="cv")
                    ssc = ssq2_ps.tile([128, NQ], F32, tag="ssc")
                    pe_ps = []
                    for m in range(6):
                        mp = 128 if m < 4 else 64
                        col = slice(128 * m, 128 * m + 128) if m < 4 else \
                            slice(512 + 64 * (m - 4), 512 + 64 * (m - 3))
                        ps = st_ps.tile([mp, NQ], F32, tag="stps")
                        for k in range(D_T):
                            nc.tensor.matmul(
                                ps,
                                kva_w[:, k, col],
                                h_t[:, k, :],
                                start=(k == 0),
                                stop=(k == D_T - 1),
                            )
                        if m < 4:
                            nc.vector.tensor_copy(cv_t[:, m, :], ps)
                            sq = sqp.tile([128, NQ], BF, tag="sq")
                            nc.scalar.activation(out=sq, in_=ps, func=AF.Square)
                            nc.tensor.matmul(
                                ssc, ones_t, sq, start=(m == 0), stop=(m == CV_T - 1)
                            )
                        else:
                            pe_ps.append(ps)

                    # ---- q_a: 12 m-tiles ----
                    qa_t = qasb.tile([128, QL_T, NQ], BF, tag="qa")
                    ssq = ssq_ps.tile([128, NQ], F32, tag="ssq")
                    for m in range(QL_T):
                        ps = st_ps.tile([128, NQ], F32, tag="stps")
                        for k in range(D_T):
                            nc.tensor.matmul(
                                ps,
                                qa_w[:, k, 128 * m : 128 * (m + 1)],
                                h_t[:, k, :],
                                start=(k == 0),
                                stop=(k == D_T - 1),
                            )
                        nc.vector.tensor_copy(qa_t[:, m, :], ps)
                        sq = sqp.tile([128, NQ], BF, tag="sq")
                        nc.scalar.activation(out=sq, in_=ps, func=AF.Square)
                        nc.tensor.matmul(
                            ssq, ones_t, sq, start=(m == 0), stop=(m == QL_T - 1)
                        )

                    # ---- kv norm + kpe rope ----
                    bc2 = nrm2.tile([128, NQ], F32, tag="bc2")
                    nc.scalar.activation(
                        out=bc2, in_=ssc, func=AF.Sqrt, scale=1.0 / KV_LORA, bias=eps_t
                    )
                    nc.vector.reciprocal(bc2, bc2)
                    for i in range(CV_T):
                        nc.vector.tensor_mul(ckvn[i][:, cs], cv_t[:, i, :], bc2)
                    t1 = pet.tile([128, NQ], F32, tag="t1")
                    t2 = pet.tile([128, NQ], F32, tag="t2")
                    nc.vector.tensor_mul(t1[0:64, :], pe_ps[0], cos_c[0:64, :])
                    nc.vector.tensor_mul(t2[0:64, :], pe_ps[1], sin_c[0:64, :])
                    nc.vector.tensor_add(kper[:, cs], t1[0:64, :], t2[0:64, :])

                    # ---- q_b: 4 col-blocks accumulated over 12 m ----
                    bc = nrm.tile([128, NQ], F32, tag="bc")
                    nc.scalar.activation(
                        out=bc, in_=ssq, func=AF.Sqrt, scale=1.0 / Q_LORA, bias=eps_t
                    )
                    nc.vector.reciprocal(bc, bc)
                    qt_tiles = []
                    for b in range(4):
                        ps = qt_ps.tile([128, NQ], F32, tag="qtps")
                        for m in range(QL_T):
                            nc.tensor.matmul(
                                ps,
                                qb_w[:, m, 128 * b : 128 * (b + 1)],
                                qa_t[:, m, :],
                                start=(m == 0),
                                stop=(m == QL_T - 1),
                            )
                        if b == 0:
                            nc.vector.tensor_mul(qn_T[0][:, cs], ps, bc)
                        elif b == 1:
                            nc.vector.tensor_mul(qn_T[1][:, cs], ps, bc)
                        else:
                            qt_tiles.append(ps)
                    nc.vector.tensor_mul(t1, qt_tiles[0], cos_c)
                    nc.vector.tensor_mul(t2, qt_tiles[1], sin_c)
                    nc.vector.tensor_add(t1, t1, t2)
                    nc.vector.tensor_mul(qpeP[:, cs], t1, bc)
                nc.sync.dma_start(out=qpe1[:, :], in_=qpeP[64:128, :])

            # ---------------- phase B2: kv_b projections ----------------
            with tc.tile_pool(name="persist2", bufs=1) as persist2:
                kn_T = [persist2.tile([128, S], BF, tag=f"knT{h}", name=f"knT{h}") for h in range(HPC)]
                v_sb = [persist2.tile([128, S], BF, tag=f"v{h}", name=f"v{h}") for h in range(HPC)]
                o_T = [persist2.tile([128, S], BF, tag=f"oT{h}", name=f"oT{h}") for h in range(HPC)]
                with tc.tile_pool(name="kbw", bufs=1) as kbw, \
                     tc.tile_pool(name="kn_ps", bufs=2, space="PSUM") as kn_ps, \
                     tc.tile_pool(name="v_ps", bufs=3, space="PSUM") as v_ps:
                    kb_w = kbw.tile([128, CV_T, 256], BF, tag="kbw")
                    vb_w = kbw.tile([128, CV_T, 256], BF, tag="vbw")
                    for ct in range(CV_T):
                        nc.sync.dma_start(out=kb_w[:, ct, :], in_=kbT[128 * ct : 128 * (ct + 1), :])
                        nc.sync.dma_start(out=vb_w[:, ct, :], in_=vbT[128 * ct : 128 * (ct + 1), :])
                    for h in range(HPC):
                        hs = slice(128 * h, 128 * (h + 1))
                        for c in range(NCHUNK):
                            cs = slice(NQ * c, NQ * (c + 1))
                            ps = kn_ps.tile([128, NQ], F32, tag="knps")
                            for ct in range(CV_T):
                                nc.tensor.matmul(
                                    ps,
                                    kb_w[:, ct, hs],
                                    ckvn[ct][:, cs],
                                    start=(ct == 0),
                                    stop=(ct == CV_T - 1),
                                )
                            nc.vector.tensor_copy(kn_T[h][:, cs], ps)
                        for kt in range(KT):
                            ks = slice(128 * kt, 128 * (kt + 1))
                            ps = v_ps.tile([128, VD], F32, tag="vps")
                            for ct in range(CV_T):
                                nc.tensor.matmul(
                                    ps,
                                    ckvn[ct][:, ks],
                                    vb_w[:, ct, hs],
                                    start=(ct == 0),
                                    stop=(ct == CV_T - 1),
                                )
                            nc.vector.tensor_copy(v_sb[h][:, ks], ps)

                # ---------------- phase C: attention ----------------
                mskp_cm = tc.tile_pool(name="mskp", bufs=1)
                oww_cm = tc.tile_pool(name="oww", bufs=1)
                mskp = mskp_cm.__enter__()
                oww = oww_cm.__enter__()
                with tc.tile_pool(name="pp", bufs=4) as pp, \
                     tc.tile_pool(name="ep", bufs=2) as ep, \
                     tc.tile_pool(name="rvp", bufs=2) as rvp, \
                     tc.tile_pool(name="ostg", bufs=4) as ostg, \
                     tc.tile_pool(name="s_ps", bufs=3, space="PSUM") as s_ps, \
                     tc.tile_pool(name="rs_ps", bufs=2, space="PSUM") as rs_ps, \
                     tc.tile_pool(name="o_ps", bufs=2, space="PSUM") as o_ps, \
                     tc.tile_pool(name="out_ps", bufs=1, space="PSUM") as out_ps:
                    mask_s = mskp.tile([128, 1024], BF, tag="mask")
                    nc.sync.dma_start(out=mask_s, in_=maskb[:, :])
                    ow_t = oww.tile([128, HPC, D], BF, tag="oww")
                    for j in range(HPC):
                        nc.sync.dma_start(out=ow_t[:, j, :], in_=owT[128 * j : 128 * (j + 1), :])
                    for c in range(NCHUNK):
                        cs = slice(NQ * c, NQ * (c + 1))
                        nkt = 4 * (c + 1)
                        for h in range(HPC):
                            qpe_h = qpeP[0:64, :] if h == 0 else qpe1
                            rs = rs_ps.tile([128, NQ], F32, tag="rs")
                            op = o_ps.tile([128, NQ], F32, tag="op")
                            for kt in range(nkt):
                                ks = slice(128 * kt, 128 * (kt + 1))
                                i = kt - 4 * c
                                lo = 128 * i if i > 0 else 0  # valid q-subrange start
                                qs = slice(NQ * c + lo, NQ * (c + 1))
                                vs = slice(lo, NQ)
                                sp = s_ps.tile([128, NQ], F32, tag="sp")
                                nc.tensor.matmul(
                                    sp[:, vs], kn_T[h][:, ks], qn_T[h][:, qs],
                                    start=True, stop=False,
                                )
                                nc.tensor.matmul(
                                    sp[:, vs], kper[:, ks], qpe_h[:, qs],
                                    start=False, stop=True,
                                )
                                p_t = pp.tile([128, NQ], BF, tag="p")
                                if kt >= 4 * c:
                                    e_t = ep.tile([128, NQ], BF, tag="e")
                                    nc.scalar.activation(out=e_t[:, vs], in_=sp[:, vs], func=AF.Exp)
                                    nc.vector.tensor_mul(
                                        p_t[:, vs], e_t[:, vs],
                                        mask_s[:, 384 : 896 - lo],
                                    )
                                else:
                                    nc.scalar.activation(out=p_t[:, vs], in_=sp[:, vs], func=AF.Exp)
                                nc.tensor.matmul(
                                    rs[:, vs], ones_t, p_t[:, vs],
                                    start=(kt == 0), stop=(kt == nkt - 1),
                                )
                                nc.tensor.matmul(
                                    op[:, vs],
                                    v_sb[h][:, ks],
                                    p_t[:, vs],
                                    start=(kt == 0), stop=(kt == nkt - 1),
                                )
                            rv = rvp.tile([128, NQ], F32, tag="rv")
                            nc.vector.reciprocal(rv, rs)
                            nc.vector.tensor_mul(o_T[h][:, cs], op, rv)
                        # o_proj for this chunk's 4 s-tiles (both heads now done;
                        # last chunk handled in a post-phase with deeper PSUM)
                        for si in range(4 * c, 4 * (c + 1) if c < NCHUNK - 1 else 4 * c):
                            ss = slice(128 * si, 128 * (si + 1))
                            for nch in range(NCHUNK):
                                ns = slice(NQ * nch, NQ * (nch + 1))
                                ps = out_ps.tile([128, NQ], F32, tag="outps")
                                for j in range(HPC):
                                    nc.tensor.matmul(
                                        ps,
                                        o_T[j][:, ss],
                                        ow_t[:, j, ns],
                                        start=(j == 0),
                                        stop=(j == HPC - 1),
                                    )
                                stg = ostg.tile([128, NQ], F32, tag="ostg")
                                nc.scalar.activation(out=stg, in_=ps, func=AF.Copy)
                                nc.sync.dma_start(out=out[ss, ns], in_=stg)
                # ---------------- final chunk o_proj ----------------
                with tc.tile_pool(name="ostg2", bufs=4) as ostg2, \
                     tc.tile_pool(name="out2_ps", bufs=4, space="PSUM") as out2_ps:
                    for si in range(4 * (NCHUNK - 1), 4 * NCHUNK):
                        ss = slice(128 * si, 128 * (si + 1))
                        for nch in range(NCHUNK):
                            ns = slice(NQ * nch, NQ * (nch + 1))
                            ps = out2_ps.tile([128, NQ], F32, tag="out2ps")
                            for j in range(HPC):
                                nc.tensor.matmul(
                                    ps,
                                    o_T[j][:, ss],
                                    ow_t[:, j, ns],
                                    start=(j == 0),
                                    stop=(j == HPC - 1),
                                )
                            stg = ostg2.tile([128, NQ], F32, tag="ostg2")
                            nc.scalar.activation(out=stg, in_=ps, func=AF.Copy)
                            nc.sync.dma_start(out=out[ss, ns], in_=stg)
                oww_cm.__exit__(None, None, None)
                mskp_cm.__exit__(None, None, None)

    _split_waits(nc)
    return nc


# ----------------------------------------------------------------------------
# entry point
# ----------------------------------------------------------------------------

def kernel(**inputs):
    global LAST_RESULTS
    shared, per_core = _prep_inputs(inputs)
    if "nc" not in _CACHE:
        _CACHE["nc"] = _build_nc()
    nc = _CACHE["nc"]
    in_maps = []
    for c in range(N_CORES):
        m = {
            "hT": shared["hT"],
            "qaT": shared["qaT"],
            "kvaT": shared["kvaT"],
            "cosb": shared["cosb"],
            "sinb": shared["sinb"],
            "maskb": shared["maskb"],
            "qbT": per_core[c]["qbT"],
            "kbT": per_core[c]["kbT"],
            "vbT": per_core[c]["vbT"],
            "owT": per_core[c]["owT"],
        }
        in_maps.append(m)
    res = run_bass_kernel_spmd(nc, in_maps, core_ids=list(range(N_CORES)))
    LAST_RESULTS = res
    out = np.zeros((S, D), dtype=np.float32)
    for r in res.results:
        out += r["out"]
    return out.reshape(B, S, D)
